# revision 62
# baseline (speedup 1.0000x reference)
"""CSPNGenerate Trainium2 kernel.

Per core (8 cores = batch b in 0..3  x  half in 0..1):
  input slab  [128p = 2 row-halves x 64ch, 11 blocks, 10 rows x 1218] bf16
  output      [9, 176, 1216] bf16   (plane p = kernel row p; plane 4 holds
               -R*T, host adds 1)

Math per pixel: C[m] = sum_t w'_t(shifted-window) contraction, m=0..8
(row 4 is the channel-sum row T, others are the 8 guide channels);
y = C + b~; S = sum_c |y_c|; R = 1/S; plane_p = y_p * R (row 4 negated).

Key structure (what makes it fast):
- Conv matmuls are emitted tap-major with an explicit 2x4 PE tiling
  (tile_position=(64h, 32g)): all EIGHT row/column tiles stream
  concurrently, retiring a full 4096-column tap octet every ~140-200ns.
  The four row-groups of each half accumulate into ONE PSUM bank at
  partition offsets 32g (per-region start=True: the has_written clear is
  per-element, not bank-wide).
- The whole normalize tail runs on the packed [105, n] layout at full
  width: one ACT op per half evacuates y = C + bias to bf16; |y| is a
  single u16 bitwise-AND on DVE (2-byte SBUF-only op -> fast mode); S and
  the R broadcast are selector matmuls (S sums at rows 0:8 and 32:40 via
  column tiles so the two broadcast matmuls pair on separate row-tiles);
  1/S is a direct InstActivation Reciprocal on ACT (the bass wrapper
  refuses it, but it shares the act table with Identity/Abs/Copy and is
  accurate to ~1e-3 here).
- bf16 everywhere off-chip (slab, weights, output) halves DMA traffic;
  selector work is software-pipelined one chunk (S/reciprocal) and two
  chunks (broadcast/multiply/output-DMA) behind the conv so the PE never
  waits at chunk boundaries.
"""

import sys

if "/opt/trn_rl_repo" not in sys.path:
    sys.path.insert(0, "/opt/trn_rl_repo")

import numpy as np
import concourse.bass as bass
import concourse.mybir as mybir
from concourse.tile import TileContext
from concourse.vector_clock import ScopedClock, VectorClock


# ---- toolchain workarounds (drain-wait split, per-instruction sync-wait
# limit, optional NTFF profiling shim) ----
def _drain_and_barrier_split(self, tick_clock, wait_clock):
    gclock = tick_clock.global_clock
    nprocs = len(gclock)
    # One NOP per nonzero proc tick; add_sem_waits elides already-observed
    # ticks, so each NOP carries at most one wait.
    for proc in range(nprocs):
        tick = gclock[proc]
        if tick <= 0:
            continue
        vc = VectorClock([0] * nprocs)
        vc.require_at_least(proc, tick)
        nop = self.nc.sync.nop(nofuse=True, hint="drain_split_wait")
        wait_clock.add_sem_waits(nop.ins, ScopedClock({None: vc}))

    # All waits were attached to the NOPs above (same engine, program order),
    # so the drain itself needs none — keeping it under the CoreV3 codegen
    # limit on sync-wait commands for Drain.
    self.nc.sync.drain()

    self.nc.all_engine_barrier()
    assert self.sems is not None
    popped = self.nc._tile_sem_poison_stack.pop()
    assert popped is self._sem_poison
    self.nc.clear_and_free_semaphores(list(self.sems.allocated().values()))
    self.nc.all_engine_barrier()


def install():
    TileContext._drain_and_barrier = _drain_and_barrier_split
    install_wait_split()


_MAX_WAITS = 1


def _split_waits_json(bir: bytes) -> bytes:
    """Walrus in this toolchain rejects instructions carrying more than one
    sync-wait command ("Too many sync wait commands"). Move excess waits onto
    same-engine NoOps inserted immediately before the instruction."""
    import orjson

    m = orjson.loads(bir)
    for func in m.get("functions", []):
        for block in func.get("blocks", []):
            out = []
            changed = False
            for inst in block["instructions"]:
                si = inst.get("sync_info") or {}
                waits = si.get("on_wait") or []
                if len(waits) > _MAX_WAITS:
                    keep = waits[-_MAX_WAITS:]
                    extra = waits[:-_MAX_WAITS]
                    for k, w in enumerate(extra):
                        out.append(
                            {
                                "debug": inst.get("debug", 0),
                                "engine": inst["engine"],
                                "ins": [],
                                "name": f"{inst['name']}-wsplit{k}",
                                "opcode": "NoOp",
                                "outs": [],
                                "sync_info": {"on_update": [], "on_wait": [w]},
                                "text_hint": "wait_split",
                            }
                        )
                    si["on_wait"] = keep
                    inst["sync_info"] = si
                    changed = True
                out.append(inst)
            if changed:
                block["instructions"] = out
    return orjson.dumps(m)


def install_wait_split():
    import concourse.bass as _bass

    if getattr(_bass.Bass, "_wait_split_installed", False):
        return
    orig = _bass.Bass.to_json_bytes

    def to_json_bytes(self):
        return _split_waits_json(orig(self))

    _bass.Bass.to_json_bytes = to_json_bytes
    _bass.Bass._wait_split_installed = True


def install_ntff_shim():
    """Provide the missing ``antenv.axon_hooks`` module so trace=True works
    under axon, wiring it to trn_boot's ctypes NTFF hook factory."""
    import sys
    import types

    if "antenv.axon_hooks" in sys.modules:
        return
    mod = types.ModuleType("antenv.axon_hooks")
    state = {"hook": None}

    def set_axon_ntff_profile_hook(h):
        state["hook"] = h

    def get_axon_ntff_profile_hook():
        return state["hook"]

    mod.set_axon_ntff_profile_hook = set_axon_ntff_profile_hook
    mod.get_axon_ntff_profile_hook = get_axon_ntff_profile_hook
    sys.modules["antenv.axon_hooks"] = mod

    try:
        from trn_agent_boot.trn_boot import _ntff_profile_via_ctypes

        hook = _ntff_profile_via_ctypes("/opt/axon/libaxon_pjrt.so")
        if hook is not None:
            set_axon_ntff_profile_hook(hook)
    except Exception as e:  # profiling optional — degrade to no trace
        print(f"ntff shim: hook install failed: {e}")

    # upload_artifacts pushes to a remote bucket that doesn't exist in this
    # container; stub it so trace post-processing stays local.
    from concourse import bass_utils

    bass_utils.upload_artifacts = lambda tmpdir: tmpdir


# geometry
B, C, H, W, K = 4, 64, 352, 1216, 3
OC = 8
HALF = 176  # rows per core
WP = W + 2  # padded width
RB = 8  # rows per half-block (per slab load)
SLAB_ROWS = RB + 2
NBLK = HALF // (2 * RB)  # 11
XCH = [(0, 256), (256, 256), (512, 256), (768, 256), (1024, 192)]
NP = 105  # packed partition extent: 3*32 + 9

F32 = mybir.dt.float32
F32R = mybir.dt.float32r
BF16 = mybir.dt.bfloat16
U32 = mybir.dt.uint32


def act_reciprocal(nc, out_ap, in_ap):
    """Direct InstActivation Reciprocal (the bass wrapper refuses it for
    accuracy reasons; our tolerance is loose and S is well-conditioned).
    Lives in the same act-table set as Identity/Abs/Copy, so no reloads."""
    sc = nc.scalar
    ins_ = [
        sc.lower_ap(in_ap),
        mybir.ImmediateValue(dtype=mybir.dt.float32, value=0.0),  # bias
        mybir.ImmediateValue(dtype=mybir.dt.float32, value=1.0),  # scale
        mybir.ImmediateValue(dtype=mybir.dt.float32, value=0.0),  # alpha
    ]
    return sc.add_instruction(
        mybir.InstActivation(
            name=sc.bass.get_next_instruction_name(),
            func=mybir.ActivationFunctionType.Reciprocal,
            ins=ins_,
            outs=[sc.lower_ap(out_ap)],
        )
    )


def build_nc():
    nc = bass.Bass()
    slab = nc.dram_tensor(
        "slab", [128, NBLK, SLAB_ROWS * WP], BF16, kind="ExternalInput"
    )
    w9 = nc.dram_tensor("w9", [128, 81], BF16, kind="ExternalInput")
    b9 = nc.dram_tensor("b9", [128, 1], F32, kind="ExternalInput")
    ssel = nc.dram_tensor("ssel", [128, 16], BF16, kind="ExternalInput")
    bsel = nc.dram_tensor("bsel", [128, 256], F32, kind="ExternalInput")
    out = nc.dram_tensor("out", [9, HALF, W], BF16, kind="ExternalOutput")

    with TileContext(nc) as tc:
        with (
            tc.tile_pool(name="consts", bufs=1) as cpool,
            tc.tile_pool(name="slabp", bufs=3) as slabp,
            tc.tile_pool(name="work", bufs=5) as work,
            tc.tile_pool(name="stagep", bufs=2) as stagep,
            tc.tile_pool(name="pc0", bufs=2, space="PSUM") as pc0,
            tc.tile_pool(name="pc1", bufs=2, space="PSUM") as pc1,
            tc.tile_pool(name="ps_s8", bufs=1, space="PSUM") as ps_s8,
            tc.tile_pool(name="ps_rbc", bufs=1, space="PSUM") as ps_rbc,
        ):
            w9t = cpool.tile([128, 81], BF16, name="w9t")
            b9t = cpool.tile([128, 1], F32, name="b9t")
            sselt = cpool.tile([128, 16], BF16, name="sselt")
            bselt = cpool.tile([128, 256], F32R, name="bselt")
            nc.gpsimd.dma_start(w9t[:], w9[:])
            nc.gpsimd.dma_start(b9t[:], b9[:])
            nc.gpsimd.dma_start(sselt[:], ssel[:])
            nc.gpsimd.dma_start(bselt[:], bsel[:])

            # Software pipeline: PE must stay dense, so each chunk's selector
            # matmuls are deferred behind the NEXT chunk's conv (stage 1:
            # S-matmul + reciprocal) and the one after (stage 2: broadcast
            # matmul + multiplies + output DMAs).
            from collections import deque

            pending = deque()

            def stage1(stt):
                s8 = ps_s8.tile([128, 512], F32, name="s8")
                n = stt["n"]
                # h=0 sums land at rows 0:8 (tile col 0), h=1 at rows 32:40
                # (tile col 32) so the two broadcast matmuls can later run on
                # separate PE row-tiles concurrently.
                for h in range(2):
                    nc.tensor.matmul(
                        out=s8[32 * h : 32 * h + 8, 0:n],
                        lhsT=sselt[0:NP, 8 * h : 8 * h + 8],
                        rhs=stt["aas"][h][0:NP, 0:n],
                        start=True,
                        stop=True,
                        tile_position=(0, 32 * h),
                        skip_group_check=(h == 1),
                    )
                r8 = work.tile([128, 512], F32R, name="r8")
                act_reciprocal(nc, r8[0:40, 0:n], s8[0:40, 0:n])
                stt["r8"] = r8

            def stage2(stt):
                n, w, x0 = stt["n"], stt["w"], stt["x0"]
                rbcs = [
                    ps_rbc.tile([128, 512], F32, name="rbcA"),
                    ps_rbc.tile([128, 512], F32, name="rbcB"),
                ]
                for h in range(2):
                    nc.tensor.matmul(
                        out=rbcs[h][0:NP, 0:n],
                        lhsT=bselt[32 * h : 32 * h + 8, 128 * h : 128 * h + NP],
                        rhs=stt["r8"][32 * h : 32 * h + 8, 0:n],
                        start=True,
                        stop=True,
                        tile_position=(32 * h, 0),
                    )
                # stage per-partition layout (h, r, x): 2 x 2 x W
                sv = stt["stage"][:].rearrange("p (h r x) -> p h r x", h=2, r=2, x=W)
                for h in range(2):
                    nc.vector.tensor_mul(
                        sv[0:NP, h, :, x0 : x0 + w],
                        stt["ys"][h][0:NP, 0:n].rearrange(
                            "p (r x) -> p r x", r=2, x=w
                        ),
                        rbcs[h][0:NP, 0:n].rearrange("p (r x) -> p r x", r=2, x=w),
                    )
                if stt["last_chunk"]:
                    blk = stt["blk"]
                    ov = out[:].rearrange(
                        "p (a h g r) w -> p a h g r w", a=NBLK, h=2, g=4, r=2
                    )
                    sb = stt["stage"][:].rearrange(
                        "p (h r x) -> p h r x", h=2, r=2, x=W
                    )
                    for g in range(4):
                        nc.sync.dma_start(
                            ov[:, blk, :, g, :, :], sb[32 * g : 32 * g + 9]
                        )

            def advance(newstate):
                if len(pending) >= 1:
                    stage1(pending[-1])
                if len(pending) >= 2:
                    stage2(pending.popleft())
                if newstate is not None:
                    pending.append(newstate)

            # host pre-splits each block: partitions 0:64 hold rows
            # [y0, y0+RB+2), partitions 64:128 hold rows [y0+RB, y0+2RB+2) —
            # one full 128-partition DMA per block for full port bandwidth.
            # Prefetched one block ahead on the gpsimd queue so the sync
            # queue's output DMAs can't delay it.
            slab_q = deque()
            for pre in range(2):
                t = slabp.tile([128, SLAB_ROWS * WP], BF16, name="st")
                nc.gpsimd.dma_start(t[:], slab[:, pre, :])
                slab_q.append(t)
            for blk in range(NBLK):
                st = slab_q.popleft()
                # 2D view: [128, SLAB_ROWS, WP]
                stv = st[:].rearrange("p (r w) -> p r w", r=SLAB_ROWS, w=WP)
                stage = stagep.tile([128, 4 * W], BF16, name="stageAB")
                for ci, (x0, w) in enumerate(XCH):
                    n = 2 * w  # elems per slot (2 rows of w)
                    cb = [
                        pc0.tile([128, 512], F32, name="c0"),
                        pc1.tile([128, 512], F32, name="c1"),
                    ]
                    cv = [
                        ct[:, 0:n].rearrange("p (r x) -> p r x", r=2, x=w)
                        for ct in cb
                    ]
                    if blk == 0 and ci < 2:
                        # PSUM may hold NaN/Inf from a previous program; the
                        # downstream ops read the never-matmul-written gap
                        # rows, so zero all conv bank buffers once.
                        for ct in cb:
                            nc.vector.memset(ct[:], 0.0)
                    # t-major emission: all 8 (g, h) column/row tiles stream
                    # per tap, one start per bank, stop on each region's last
                    # tap.
                    for t in range(9):
                        di, dj = t // 3, t % 3
                        for g in range(4):
                            for h in range(2):
                                nc.tensor.matmul(
                                    out=cv[h][32 * g : 32 * g + 9, :, :],
                                    lhsT=w9t[
                                        64 * h : 64 * h + 64, 9 * t : 9 * t + 9
                                    ],
                                    rhs=stv[
                                        64 * h : 64 * h + 64,
                                        2 * g + di : 2 * g + di + 2,
                                        x0 + dj : x0 + dj + w,
                                    ],
                                    start=(t == 0),
                                    stop=(t == 8),
                                    tile_position=(64 * h, 32 * g),
                                    skip_group_check=not (t == 0 and g == 0),
                                )

                    newstate = {
                        "n": n,
                        "w": w,
                        "x0": x0,
                        "stage": stage,
                        "last_chunk": ci == len(XCH) - 1,
                        "blk": blk,
                        "cb": cb,
                    }
                    aas = [
                        work.tile([128, 512], BF16, name="aA"),
                        work.tile([128, 512], BF16, name="aB"),
                    ]
                    ys = [
                        work.tile([128, 512], BF16, name="yA"),
                        work.tile([128, 512], BF16, name="yB"),
                    ]
                    newstate["aas"] = aas
                    newstate["ys"] = ys

                    # evacuate y = C + bias as bf16 on ACT.  Emitted BEFORE
                    # the deferred selector work so the evacs lead the ACT
                    # queue: the next-next chunk's conv reuses these banks,
                    # and a late evac stalls the whole PE pipeline.
                    for h in range(2):
                        nc.scalar.activation(
                            ys[h][0:NP, 0:n],
                            cb[h][0:NP, 0:n],
                            mybir.ActivationFunctionType.Identity,
                            bias=b9t[0:NP, 0:1],
                            scale=1.0,
                        )
                    advance(newstate)

                    # |y| = clear the bf16 sign bit; SBUF-only 2-byte DVE op
                    # runs in the fast 2x mode.  After advance() so the
                    # previous chunks' multiplies lead the DVE queue.
                    for h in range(2):
                        nc.vector.tensor_scalar(
                            aas[h][0:NP, 0:n].bitcast(mybir.dt.uint16),
                            ys[h][0:NP, 0:n].bitcast(mybir.dt.uint16),
                            0x7FFF,
                            None,
                            mybir.AluOpType.bitwise_and,
                        )

                    if ci == 0 and blk + 2 < NBLK:
                        t = slabp.tile([128, SLAB_ROWS * WP], BF16, name="st")
                        nc.gpsimd.dma_start(t[:], slab[:, blk + 2, :])
                        slab_q.append(t)
            # drain the pipeline
            for stt in pending:
                if "r8" not in stt:
                    stage1(stt)
            while pending:
                stage2(pending.popleft())
    return nc


def make_consts(conv_w, gamma, beta, mean, var):
    eps = 1e-5
    s = gamma.astype(np.float64) / np.sqrt(var.astype(np.float64) + eps)
    bt = beta.astype(np.float64) - mean.astype(np.float64) * s
    wp = conv_w.astype(np.float64) * s[:, None, None, None]  # [8, 64, 3, 3]

    # kernel row order: plane p <-> row p (row 4 = channel-sum row T)
    # row r < 4 -> channel r; row 4 -> sum; row r > 4 -> channel r-1
    w9 = np.zeros((128, 81), np.float32)
    for h in range(2):
        for t in range(9):
            di, dj = t // 3, t % 3
            blk = wp[:, :, di, dj]  # [oc, c]
            cols = w9[64 * h : 64 * h + 64, 9 * t : 9 * t + 9]
            cols[:, 0:4] = blk[0:4].T
            cols[:, 4] = blk.sum(axis=0)
            cols[:, 5:9] = blk[4:8].T
    b9 = np.zeros((128, 1), np.float32)
    bt9 = np.zeros(9)
    bt9[0:4] = bt[0:4]
    bt9[4] = bt.sum()
    bt9[5:9] = bt[4:8]
    for g in range(4):
        b9[32 * g : 32 * g + 9, 0] = bt9

    # S-selector: sum |y| over the 8 guide-channel rows (skip row 4 = T).
    # pass h=0 (cols 0:8): group g -> S row g; pass h=1 (cols 8:16): -> 4+g
    ch_rows = [0, 1, 2, 3, 5, 6, 7, 8]
    ssel = np.zeros((128, 16), np.float32)  # cast to bf16 at the call site
    for g in range(4):
        for r in ch_rows:
            ssel[32 * g + r, g] = 1.0
            ssel[32 * g + r, 8 + 4 + g] = 1.0
    # broadcast selector: R row (4h + g) -> packed rows 32g+r (col block h);
    # +1 for guide rows, -1 for the T row (plane 4 = -T*R, host adds 1)
    # h0 rows at partitions 0:8 (S rows 0:8 -> g), h1 at 32:40 (-> 36+g)
    bsel = np.zeros((128, 256), np.float32)
    for g in range(4):
        for r in range(9):
            v = -1.0 if r == 4 else 1.0
            bsel[g, 32 * g + r] = v
            bsel[36 + g, 128 + 32 * g + r] = v
    return w9, b9, ssel, bsel


TRACE = False
LAST_EXEC_NS = None


def kernel(feature, conv_w, gamma, beta, mean, var, kernel_size):
    global LAST_EXEC_NS
    install()
    if TRACE:
        install_ntff_shim()

    from concourse.bass_utils import run_bass_kernel_spmd

    import ml_dtypes

    feature = np.asarray(feature, np.float32)
    conv_w = np.asarray(conv_w, np.float32)
    gamma = np.asarray(gamma, np.float32)
    beta = np.asarray(beta, np.float32)
    mean = np.asarray(mean, np.float32)
    var = np.asarray(var, np.float32)

    w9, b9, ssel, bsel = make_consts(conv_w, gamma, beta, mean, var)
    w9 = w9.astype(ml_dtypes.bfloat16)
    ssel = ssel.astype(ml_dtypes.bfloat16)

    # padded feature [B, C, H+2, W+2]
    fpad = np.zeros((B, C, H + 2, WP), ml_dtypes.bfloat16)
    fpad[:, :, 1 : H + 1, 1 : W + 1] = feature

    in_maps = []
    for core in range(8):
        b, half = core // 2, core % 2
        h0 = half * HALF
        # per-block slab: partitions 0:64 = channels for rows [y0, y0+10),
        # partitions 64:128 = channels for rows [y0+8, y0+18)
        slab2 = np.empty((128, NBLK, SLAB_ROWS * WP), ml_dtypes.bfloat16)
        for k in range(NBLK):
            r0 = h0 + 2 * RB * k
            slab2[0:64, k, :] = fpad[b, :, r0 : r0 + SLAB_ROWS, :].reshape(C, -1)
            slab2[64:128, k, :] = fpad[b, :, r0 + RB : r0 + RB + SLAB_ROWS, :].reshape(
                C, -1
            )
        in_maps.append(
            {
                "slab": slab2,
                "w9": w9,
                "b9": b9,
                "ssel": ssel,
                "bsel": bsel,
            }
        )

    nc = build_nc()
    res = run_bass_kernel_spmd(nc, in_maps, core_ids=list(range(8)), trace=TRACE)
    LAST_EXEC_NS = res.exec_time_ns

    out_full = np.zeros((B, 9, H + 2, WP), np.float32)
    for core in range(8):
        b, half = core // 2, core % 2
        h0 = half * HALF
        r = np.asarray(res.results[core]["out"], np.float32)  # [9, 176, 1216]
        for p in range(9):
            i, j = p // 3, p % 3
            plane = r[p]
            if p == 4:
                plane = 1.0 + plane
            out_full[b, p, h0 + i : h0 + HALF + i, j : j + W] = plane
    return out_full


# revision 68
# speedup vs baseline: 1.0337x; 1.0337x over previous
"""CSPNGenerate Trainium2 kernel.

Per core (8 cores = batch b in 0..3  x  half in 0..1):
  input slab  [128p = 2 row-halves x 64ch, 11 blocks, 10 rows x 1218] bf16
  output      [9, 176, 1216] bf16   (plane p = kernel row p; plane 4 holds
               -R*T, host adds 1)

Math per pixel: C[m] = sum_t w'_t(shifted-window) contraction, m=0..8
(row 4 is the channel-sum row T, others are the 8 guide channels);
y = C + b~; S = sum_c |y_c|; R = 1/S; plane_p = y_p * R (row 4 negated).

Key structure (what makes it fast):
- Conv matmuls are emitted tap-major with an explicit 2x4 PE tiling
  (tile_position=(64h, 32g)): all EIGHT row/column tiles stream
  concurrently, retiring a full 4096-column tap octet every ~140-200ns.
  The four row-groups of each half accumulate into ONE PSUM bank at
  partition offsets 32g (per-region start=True: the has_written clear is
  per-element, not bank-wide).
- The whole normalize tail runs on the packed [105, n] layout at full
  width: one ACT op per half evacuates y = C + bias to bf16; |y| is a
  single u16 bitwise-AND on DVE (2-byte SBUF-only op -> fast mode); S and
  the R broadcast are selector matmuls (S sums at rows 0:8 and 32:40 via
  column tiles so the two broadcast matmuls pair on separate row-tiles);
  1/S is a direct InstActivation Reciprocal on ACT (the bass wrapper
  refuses it, but it shares the act table with Identity/Abs/Copy and is
  accurate to ~1e-3 here).
- bf16 everywhere off-chip (slab, weights, output) halves DMA traffic;
  selector work is software-pipelined one chunk (S/reciprocal) and two
  chunks (broadcast/multiply/output-DMA) behind the conv so the PE never
  waits at chunk boundaries.
"""

import sys

if "/opt/trn_rl_repo" not in sys.path:
    sys.path.insert(0, "/opt/trn_rl_repo")

import numpy as np
import concourse.bass as bass
import concourse.mybir as mybir
from concourse.tile import TileContext
from concourse.vector_clock import ScopedClock, VectorClock


# ---- toolchain workarounds (drain-wait split, per-instruction sync-wait
# limit, optional NTFF profiling shim) ----
def _drain_and_barrier_split(self, tick_clock, wait_clock):
    gclock = tick_clock.global_clock
    nprocs = len(gclock)
    # One NOP per nonzero proc tick; add_sem_waits elides already-observed
    # ticks, so each NOP carries at most one wait.
    for proc in range(nprocs):
        tick = gclock[proc]
        if tick <= 0:
            continue
        vc = VectorClock([0] * nprocs)
        vc.require_at_least(proc, tick)
        nop = self.nc.sync.nop(nofuse=True, hint="drain_split_wait")
        wait_clock.add_sem_waits(nop.ins, ScopedClock({None: vc}))

    # All waits were attached to the NOPs above (same engine, program order),
    # so the drain itself needs none — keeping it under the CoreV3 codegen
    # limit on sync-wait commands for Drain.
    self.nc.sync.drain()

    self.nc.all_engine_barrier()
    assert self.sems is not None
    popped = self.nc._tile_sem_poison_stack.pop()
    assert popped is self._sem_poison
    self.nc.clear_and_free_semaphores(list(self.sems.allocated().values()))
    self.nc.all_engine_barrier()


def install():
    TileContext._drain_and_barrier = _drain_and_barrier_split
    install_wait_split()


_MAX_WAITS = 1


def _split_waits_json(bir: bytes) -> bytes:
    """Walrus in this toolchain rejects instructions carrying more than one
    sync-wait command ("Too many sync wait commands"). Move excess waits onto
    same-engine NoOps inserted immediately before the instruction."""
    import orjson

    m = orjson.loads(bir)
    for func in m.get("functions", []):
        for block in func.get("blocks", []):
            out = []
            changed = False
            for inst in block["instructions"]:
                si = inst.get("sync_info") or {}
                waits = si.get("on_wait") or []
                if len(waits) > _MAX_WAITS:
                    keep = waits[-_MAX_WAITS:]
                    extra = waits[:-_MAX_WAITS]
                    for k, w in enumerate(extra):
                        out.append(
                            {
                                "debug": inst.get("debug", 0),
                                "engine": inst["engine"],
                                "ins": [],
                                "name": f"{inst['name']}-wsplit{k}",
                                "opcode": "NoOp",
                                "outs": [],
                                "sync_info": {"on_update": [], "on_wait": [w]},
                                "text_hint": "wait_split",
                            }
                        )
                    si["on_wait"] = keep
                    inst["sync_info"] = si
                    changed = True
                out.append(inst)
            if changed:
                block["instructions"] = out
    return orjson.dumps(m)


def install_wait_split():
    import concourse.bass as _bass

    if getattr(_bass.Bass, "_wait_split_installed", False):
        return
    orig = _bass.Bass.to_json_bytes

    def to_json_bytes(self):
        return _split_waits_json(orig(self))

    _bass.Bass.to_json_bytes = to_json_bytes
    _bass.Bass._wait_split_installed = True


def install_ntff_shim():
    """Provide the missing ``antenv.axon_hooks`` module so trace=True works
    under axon, wiring it to trn_boot's ctypes NTFF hook factory."""
    import sys
    import types

    if "antenv.axon_hooks" in sys.modules:
        return
    mod = types.ModuleType("antenv.axon_hooks")
    state = {"hook": None}

    def set_axon_ntff_profile_hook(h):
        state["hook"] = h

    def get_axon_ntff_profile_hook():
        return state["hook"]

    mod.set_axon_ntff_profile_hook = set_axon_ntff_profile_hook
    mod.get_axon_ntff_profile_hook = get_axon_ntff_profile_hook
    sys.modules["antenv.axon_hooks"] = mod

    try:
        from trn_agent_boot.trn_boot import _ntff_profile_via_ctypes

        hook = _ntff_profile_via_ctypes("/opt/axon/libaxon_pjrt.so")
        if hook is not None:
            set_axon_ntff_profile_hook(hook)
    except Exception as e:  # profiling optional — degrade to no trace
        print(f"ntff shim: hook install failed: {e}")

    # upload_artifacts pushes to a remote bucket that doesn't exist in this
    # container; stub it so trace post-processing stays local.
    from concourse import bass_utils

    bass_utils.upload_artifacts = lambda tmpdir: tmpdir


# geometry
B, C, H, W, K = 4, 64, 352, 1216, 3
OC = 8
HALF = 176  # rows per core
WP = W + 2  # padded width
RB = 8  # rows per half-block (per slab load)
SLAB_ROWS = RB + 2
NBLK = HALF // (2 * RB)  # 11
XCH = [(0, 256), (256, 256), (512, 256), (768, 256), (1024, 192)]
NP = 105  # packed partition extent: 3*32 + 9

F32 = mybir.dt.float32
F32R = mybir.dt.float32r
BF16 = mybir.dt.bfloat16
U32 = mybir.dt.uint32


def act_reciprocal(nc, out_ap, in_ap):
    """Direct InstActivation Reciprocal (the bass wrapper refuses it for
    accuracy reasons; our tolerance is loose and S is well-conditioned).
    Lives in the same act-table set as Identity/Abs/Copy, so no reloads."""
    sc = nc.scalar
    ins_ = [
        sc.lower_ap(in_ap),
        mybir.ImmediateValue(dtype=mybir.dt.float32, value=0.0),  # bias
        mybir.ImmediateValue(dtype=mybir.dt.float32, value=1.0),  # scale
        mybir.ImmediateValue(dtype=mybir.dt.float32, value=0.0),  # alpha
    ]
    return sc.add_instruction(
        mybir.InstActivation(
            name=sc.bass.get_next_instruction_name(),
            func=mybir.ActivationFunctionType.Reciprocal,
            ins=ins_,
            outs=[sc.lower_ap(out_ap)],
        )
    )


def build_nc():
    nc = bass.Bass()
    slab = nc.dram_tensor(
        "slab", [128, NBLK, SLAB_ROWS * WP], BF16, kind="ExternalInput"
    )
    w9 = nc.dram_tensor("w9", [128, 81], BF16, kind="ExternalInput")
    b9 = nc.dram_tensor("b9", [128, 1], F32, kind="ExternalInput")
    ssel = nc.dram_tensor("ssel", [128, 16], BF16, kind="ExternalInput")
    bsel = nc.dram_tensor("bsel", [128, 256], F32, kind="ExternalInput")
    out = nc.dram_tensor("out", [9, HALF, W], BF16, kind="ExternalOutput")

    with TileContext(nc) as tc:
        with (
            tc.tile_pool(name="consts", bufs=1) as cpool,
            tc.tile_pool(name="slabp", bufs=3) as slabp,
            tc.tile_pool(name="work", bufs=5) as work,
            tc.tile_pool(name="stagep", bufs=2) as stagep,
            tc.tile_pool(name="pcc", bufs=2, space="PSUM") as pcc,
            tc.tile_pool(name="ps_s8", bufs=1, space="PSUM") as ps_s8,
            tc.tile_pool(name="ps_rbc", bufs=1, space="PSUM") as ps_rbc,
        ):
            w9t = cpool.tile([128, 81], BF16, name="w9t")
            b9t = cpool.tile([128, 1], F32, name="b9t")
            sselt = cpool.tile([128, 16], BF16, name="sselt")
            bselt = cpool.tile([128, 256], F32R, name="bselt")
            nc.gpsimd.dma_start(w9t[:], w9[:])
            nc.gpsimd.dma_start(b9t[:], b9[:])
            nc.gpsimd.dma_start(sselt[:], ssel[:])
            nc.gpsimd.dma_start(bselt[:], bsel[:])

            # Software pipeline: PE must stay dense, so each chunk's selector
            # matmuls are deferred behind the NEXT chunk's conv (stage 1:
            # S-matmul + reciprocal) and the one after (stage 2: broadcast
            # matmul + multiplies + output DMAs).
            from collections import deque

            pending = deque()

            def stage1(stt):
                s8 = ps_s8.tile([128, 512], F32, name="s8")
                n = stt["n"]
                # h=0 sums land at rows 0:8 (tile col 0), h=1 at rows 32:40
                # (tile col 32) so the two broadcast matmuls can later run on
                # separate PE row-tiles concurrently.
                for h in range(2):
                    nc.tensor.matmul(
                        out=s8[32 * h : 32 * h + 8, 0:n],
                        lhsT=sselt[0:NP, 8 * h : 8 * h + 8],
                        rhs=stt["aas"][0:NP, 512 * h : 512 * h + n],
                        start=True,
                        stop=True,
                        tile_position=(0, 32 * h),
                        skip_group_check=(h == 1),
                    )
                r8 = work.tile([128, 512], F32R, name="r8")
                act_reciprocal(nc, r8[0:40, 0:n], s8[0:40, 0:n])
                stt["r8"] = r8

            def stage2(stt):
                n, w, x0 = stt["n"], stt["w"], stt["x0"]
                rbcs = [
                    ps_rbc.tile([128, 512], F32, name="rbcA"),
                    ps_rbc.tile([128, 512], F32, name="rbcB"),
                ]
                for h in range(2):
                    nc.tensor.matmul(
                        out=rbcs[h][0:NP, 0:n],
                        lhsT=bselt[32 * h : 32 * h + 8, 128 * h : 128 * h + NP],
                        rhs=stt["r8"][32 * h : 32 * h + 8, 0:n],
                        start=True,
                        stop=True,
                        tile_position=(32 * h, 0),
                    )
                # stage per-partition layout (h, r, x): 2 x 2 x W
                sv = stt["stage"][:].rearrange("p (h r x) -> p h r x", h=2, r=2, x=W)
                for h in range(2):
                    nc.vector.tensor_mul(
                        sv[0:NP, h, :, x0 : x0 + w],
                        stt["ys"][0:NP, 512 * h : 512 * h + n].rearrange(
                            "p (r x) -> p r x", r=2, x=w
                        ),
                        rbcs[h][0:NP, 0:n].rearrange("p (r x) -> p r x", r=2, x=w),
                    )
                if stt["last_chunk"]:
                    blk = stt["blk"]
                    ov = out[:].rearrange(
                        "p (a h g r) w -> p a h g r w", a=NBLK, h=2, g=4, r=2
                    )
                    sb = stt["stage"][:].rearrange(
                        "p (h r x) -> p h r x", h=2, r=2, x=W
                    )
                    for g in range(4):
                        nc.sync.dma_start(
                            ov[:, blk, :, g, :, :], sb[32 * g : 32 * g + 9]
                        )

            def advance(newstate):
                if len(pending) >= 1:
                    stage1(pending[-1])
                if len(pending) >= 2:
                    stage2(pending.popleft())
                if newstate is not None:
                    pending.append(newstate)

            # host pre-splits each block: partitions 0:64 hold rows
            # [y0, y0+RB+2), partitions 64:128 hold rows [y0+RB, y0+2RB+2) —
            # one full 128-partition DMA per block for full port bandwidth.
            # Prefetched one block ahead on the gpsimd queue so the sync
            # queue's output DMAs can't delay it.
            slab_q = deque()
            for pre in range(2):
                t = slabp.tile([128, SLAB_ROWS * WP], BF16, name="st")
                nc.gpsimd.dma_start(t[:], slab[:, pre, :])
                slab_q.append(t)
            for blk in range(NBLK):
                st = slab_q.popleft()
                # 2D view: [128, SLAB_ROWS, WP]
                stv = st[:].rearrange("p (r w) -> p r w", r=SLAB_ROWS, w=WP)
                stage = stagep.tile([128, 4 * W], BF16, name="stageAB")
                for ci, (x0, w) in enumerate(XCH):
                    n = 2 * w  # elems per slot (2 rows of w)
                    # one 2-bank tile: h0 accumulates in cols 0:512 (bank A),
                    # h1 in 512:1024 (bank B) -> the whole chunk evacuates in
                    # a single wide ACT op
                    cb = pcc.tile([128, 1024], F32, name="cc")
                    cv = [
                        cb[:, 512 * h : 512 * h + n].rearrange(
                            "p (r x) -> p r x", r=2, x=w
                        )
                        for h in range(2)
                    ]
                    if blk == 0 and ci < 2:
                        # PSUM may hold NaN/Inf from a previous program; the
                        # downstream ops read the never-matmul-written gap
                        # rows, so zero all conv bank buffers once.
                        for h in range(2):
                            nc.vector.memset(cb[:, 512 * h : 512 * h + 512], 0.0)
                    # t-major emission: all 8 (g, h) column/row tiles stream
                    # per tap, one start per bank, stop on each region's last
                    # tap.
                    for t in range(9):
                        di, dj = t // 3, t % 3
                        for g in range(4):
                            for h in range(2):
                                nc.tensor.matmul(
                                    out=cv[h][32 * g : 32 * g + 9, :, :],
                                    lhsT=w9t[
                                        64 * h : 64 * h + 64, 9 * t : 9 * t + 9
                                    ],
                                    rhs=stv[
                                        64 * h : 64 * h + 64,
                                        2 * g + di : 2 * g + di + 2,
                                        x0 + dj : x0 + dj + w,
                                    ],
                                    start=(t == 0),
                                    stop=(t == 8),
                                    tile_position=(64 * h, 32 * g),
                                    skip_group_check=not (
                                        t == 0 and g == 0 and h == 0
                                    ),
                                )

                    newstate = {
                        "n": n,
                        "w": w,
                        "x0": x0,
                        "stage": stage,
                        "last_chunk": ci == len(XCH) - 1,
                        "blk": blk,
                        "cb": cb,
                    }
                    aas = work.tile([128, 1024], BF16, name="aas")
                    ys = work.tile([128, 1024], BF16, name="ys")
                    newstate["aas"] = aas
                    newstate["ys"] = ys

                    # evacuate y = C + bias as bf16 in ONE wide ACT op over
                    # both banks.  Emitted BEFORE the deferred selector work
                    # so the evac leads the ACT queue: the next-next chunk's
                    # conv reuses these banks, and a late evac stalls the
                    # whole PE pipeline.
                    nc.scalar.activation(
                        ys[0:NP, :],
                        cb[0:NP, :],
                        mybir.ActivationFunctionType.Identity,
                        bias=b9t[0:NP, 0:1],
                        scale=1.0,
                    )
                    advance(newstate)

                    # |y| = clear the bf16 sign bit; one SBUF-only 2-byte DVE
                    # op (fast 2x mode).  After advance() so the previous
                    # chunks' multiplies lead the DVE queue.
                    nc.vector.tensor_scalar(
                        aas[0:NP, :].bitcast(mybir.dt.uint16),
                        ys[0:NP, :].bitcast(mybir.dt.uint16),
                        0x7FFF,
                        None,
                        mybir.AluOpType.bitwise_and,
                    )

                    if ci == 0 and blk + 2 < NBLK:
                        t = slabp.tile([128, SLAB_ROWS * WP], BF16, name="st")
                        nc.gpsimd.dma_start(t[:], slab[:, blk + 2, :])
                        slab_q.append(t)
            # drain the pipeline
            for stt in pending:
                if "r8" not in stt:
                    stage1(stt)
            while pending:
                stage2(pending.popleft())
    return nc


def make_consts(conv_w, gamma, beta, mean, var):
    eps = 1e-5
    s = gamma.astype(np.float64) / np.sqrt(var.astype(np.float64) + eps)
    bt = beta.astype(np.float64) - mean.astype(np.float64) * s
    wp = conv_w.astype(np.float64) * s[:, None, None, None]  # [8, 64, 3, 3]

    # kernel row order: plane p <-> row p (row 4 = channel-sum row T)
    # row r < 4 -> channel r; row 4 -> sum; row r > 4 -> channel r-1
    w9 = np.zeros((128, 81), np.float32)
    for h in range(2):
        for t in range(9):
            di, dj = t // 3, t % 3
            blk = wp[:, :, di, dj]  # [oc, c]
            cols = w9[64 * h : 64 * h + 64, 9 * t : 9 * t + 9]
            cols[:, 0:4] = blk[0:4].T
            cols[:, 4] = blk.sum(axis=0)
            cols[:, 5:9] = blk[4:8].T
    b9 = np.zeros((128, 1), np.float32)
    bt9 = np.zeros(9)
    bt9[0:4] = bt[0:4]
    bt9[4] = bt.sum()
    bt9[5:9] = bt[4:8]
    for g in range(4):
        b9[32 * g : 32 * g + 9, 0] = bt9

    # S-selector: sum |y| over the 8 guide-channel rows (skip row 4 = T).
    # pass h=0 (cols 0:8): group g -> S row g; pass h=1 (cols 8:16): -> 4+g
    ch_rows = [0, 1, 2, 3, 5, 6, 7, 8]
    ssel = np.zeros((128, 16), np.float32)  # cast to bf16 at the call site
    for g in range(4):
        for r in ch_rows:
            ssel[32 * g + r, g] = 1.0
            ssel[32 * g + r, 8 + 4 + g] = 1.0
    # broadcast selector: R row (4h + g) -> packed rows 32g+r (col block h);
    # +1 for guide rows, -1 for the T row (plane 4 = -T*R, host adds 1)
    # h0 rows at partitions 0:8 (S rows 0:8 -> g), h1 at 32:40 (-> 36+g)
    bsel = np.zeros((128, 256), np.float32)
    for g in range(4):
        for r in range(9):
            v = -1.0 if r == 4 else 1.0
            bsel[g, 32 * g + r] = v
            bsel[36 + g, 128 + 32 * g + r] = v
    return w9, b9, ssel, bsel


TRACE = False
LAST_EXEC_NS = None


def kernel(feature, conv_w, gamma, beta, mean, var, kernel_size):
    global LAST_EXEC_NS
    install()
    if TRACE:
        install_ntff_shim()

    from concourse.bass_utils import run_bass_kernel_spmd

    import ml_dtypes

    feature = np.asarray(feature, np.float32)
    conv_w = np.asarray(conv_w, np.float32)
    gamma = np.asarray(gamma, np.float32)
    beta = np.asarray(beta, np.float32)
    mean = np.asarray(mean, np.float32)
    var = np.asarray(var, np.float32)

    w9, b9, ssel, bsel = make_consts(conv_w, gamma, beta, mean, var)
    w9 = w9.astype(ml_dtypes.bfloat16)
    ssel = ssel.astype(ml_dtypes.bfloat16)

    # padded feature [B, C, H+2, W+2]
    fpad = np.zeros((B, C, H + 2, WP), ml_dtypes.bfloat16)
    fpad[:, :, 1 : H + 1, 1 : W + 1] = feature

    in_maps = []
    for core in range(8):
        b, half = core // 2, core % 2
        h0 = half * HALF
        # per-block slab: partitions 0:64 = channels for rows [y0, y0+10),
        # partitions 64:128 = channels for rows [y0+8, y0+18)
        slab2 = np.empty((128, NBLK, SLAB_ROWS * WP), ml_dtypes.bfloat16)
        for k in range(NBLK):
            r0 = h0 + 2 * RB * k
            slab2[0:64, k, :] = fpad[b, :, r0 : r0 + SLAB_ROWS, :].reshape(C, -1)
            slab2[64:128, k, :] = fpad[b, :, r0 + RB : r0 + RB + SLAB_ROWS, :].reshape(
                C, -1
            )
        in_maps.append(
            {
                "slab": slab2,
                "w9": w9,
                "b9": b9,
                "ssel": ssel,
                "bsel": bsel,
            }
        )

    nc = build_nc()
    res = run_bass_kernel_spmd(nc, in_maps, core_ids=list(range(8)), trace=TRACE)
    LAST_EXEC_NS = res.exec_time_ns

    out_full = np.zeros((B, 9, H + 2, WP), np.float32)
    for core in range(8):
        b, half = core // 2, core % 2
        h0 = half * HALF
        r = np.asarray(res.results[core]["out"], np.float32)  # [9, 176, 1216]
        for p in range(9):
            i, j = p // 3, p % 3
            plane = r[p]
            if p == 4:
                plane = 1.0 + plane
            out_full[b, p, h0 + i : h0 + HALF + i, j : j + W] = plane
    return out_full
